# revision 7
# baseline (speedup 1.0000x reference)
"""ConvCrossAttention Trainium2 kernel (Bass/Tile), SPMD over 8 NeuronCores.

Sharding: pure data-parallel over batch (B=16 -> 2 images per core). Each core
runs the full two-stream cross-attention block for its 2 images; no collectives.

v3. bf16 matmul path (fp32 PSUM accumulation) with the work split to balance
engines (scalar_tensor_tensor has no 2x DVE uop, so depthwise on DVE runs 1x):
  - depthwise convs run on the PE as 9 accumulating diagonal-weight matmuls
    over a host-padded 34x34 zero-border x (dense strided rhs views, stride-2
    views for the KV path). One of the four Q-side (stream, chunk) units stays
    on the DVE (via a column-shifted x copy so its taps are step-1 aligned) to
    keep PE/DVE occupancy balanced at ~105us each.
  - attention dots are K=64 row-tiled pairs (tile_position auto-derived from
    base_partition 0/64) so head pairs run concurrently in the PE array;
    softmax denominator and attn@v use M=64 col-tiled pairs (PSUM partition
    slices 0:64 / 64:128).
  - exp on ACT straight off the dots PSUM into bf16 pT; denominators
    reciprocal'd on DVE; normalization fused into the attn@v eviction.
  - image 1's depthwise+projections are emitted interleaved with image 0's
    attention blocks so DVE/PE/ACT phases overlap instead of serializing.
  - output is written bf16 (host upcasts); K-proj bias eviction on ACT,
    Q-proj/out-conv bias evictions on DVE.
"""

import numpy as np
import ml_dtypes
from contextlib import ExitStack

import concourse.bass as bass
import concourse.bacc as bacc
import concourse.tile as tile
import concourse.mybir as mybir
from concourse.bass_utils import run_bass_kernel_spmd

F32 = mybir.dt.float32
BF16 = mybir.dt.bfloat16
AOP = mybir.AluOpType
AF = mybir.ActivationFunctionType

N_CORES = 8
B = 16
IMGS = B // N_CORES          # images per core
DIM = 256                    # conv in channels
HEADS, DH, INNER = 8, 64, 512
HW = 1024                    # 32*32 q positions
HWK = 256                    # 16*16 kv positions
EPS = 1e-5
SCALE = DH ** -0.5
PADW = 34                    # zero-padded spatial extent
PADF = PADW * PADW           # 1156

BF = ml_dtypes.bfloat16

DW_DVE = {(1, 1)}            # Q-depthwise (stream, chunk) units kept on DVE


# ---------------------------------------------------------------------------
# device kernel
# ---------------------------------------------------------------------------

def _build_module():
    nc = bacc.Bacc("TRN2", target_bir_lowering=False, debug=False)

    xp_d = [nc.dram_tensor(f"xp{s}", [IMGS, 128, 2, PADF], BF16, kind="ExternalInput")
            for s in range(2)]
    xq_d = nc.dram_tensor("xq1", [IMGS, 128, PADF], BF16, kind="ExternalInput")
    dqs_d = nc.dram_tensor("dqs1", [128, 9], F32, kind="ExternalInput")
    gq_d = [nc.dram_tensor(f"gq{s}", [128, 2, 9, 128], BF16, kind="ExternalInput")
            for s in range(2)]
    gkv_d = [nc.dram_tensor(f"gkv{s}", [128, 2, 9, 128], BF16, kind="ExternalInput")
             for s in range(2)]
    wq_d = [nc.dram_tensor(f"wq{s}", [128, 2, 512], BF16, kind="ExternalInput")
            for s in range(2)]
    wkv_d = [nc.dram_tensor(f"wkv{s}", [128, 2, 1024], BF16, kind="ExternalInput")
             for s in range(2)]
    wo_d = [nc.dram_tensor(f"wo{s}", [128, 4, 256], BF16, kind="ExternalInput")
            for s in range(2)]
    bq_d = [nc.dram_tensor(f"bq{s}", [128, 4], F32, kind="ExternalInput")
            for s in range(2)]
    bk_d = [nc.dram_tensor(f"bk{s}", [128, 4], F32, kind="ExternalInput")
            for s in range(2)]
    bo_d = [nc.dram_tensor(f"bo{s}", [128, 2], F32, kind="ExternalInput")
            for s in range(2)]
    ones_d = nc.dram_tensor("ones_in", [128, 64], BF16, kind="ExternalInput")
    out_d = nc.dram_tensor("out", [2, IMGS, DIM, HW], BF16, kind="ExternalOutput")

    with tile.TileContext(nc) as tc, ExitStack() as ctx:
        const = ctx.enter_context(tc.tile_pool(name="const", bufs=1))
        xpool = ctx.enter_context(tc.tile_pool(name="xpool", bufs=6))
        yqpool = ctx.enter_context(tc.tile_pool(name="yqpool", bufs=6))
        ykpool = ctx.enter_context(tc.tile_pool(name="ykpool", bufs=6))
        qTpool = ctx.enter_context(tc.tile_pool(name="qTpool", bufs=16))
        kTpool = ctx.enter_context(tc.tile_pool(name="kTpool", bufs=16))
        vpool = ctx.enter_context(tc.tile_pool(name="vpool", bufs=8))
        pTpool = ctx.enter_context(tc.tile_pool(name="pTpool", bufs=8))
        Rpool = ctx.enter_context(tc.tile_pool(name="Rpool", bufs=4))
        oTpool = ctx.enter_context(tc.tile_pool(name="oTpool", bufs=6))
        outpool = ctx.enter_context(tc.tile_pool(name="outpool", bufs=3))
        psA = ctx.enter_context(tc.tile_pool(name="psA", bufs=2, space="PSUM"))
        psB = ctx.enter_context(tc.tile_pool(name="psB", bufs=2, space="PSUM"))
        psD = ctx.enter_context(tc.tile_pool(name="psD", bufs=2, space="PSUM"))

        # ---- constants + x loads (order = DMA priority) --------------------
        gq_sb, gkv_sb, wq_sb, wkv_sb, wo_sb, bq_sb, bk_sb, bo_sb = (
            [], [], [], [], [], [], [], [])
        for s in range(2):
            gkv_sb.append(const.tile([128, 2, 9, 128], BF16, tag=f"gkv{s}",
                                     name=f"gkv{s}"))
            gq_sb.append(const.tile([128, 2, 9, 128], BF16, tag=f"gq{s}",
                                    name=f"gq{s}"))
        xt = {}
        # first image's x + stream-0 diag weights lead the queue so the PE can
        # start depthwise almost immediately
        nc.sync.dma_start(out=gkv_sb[0][:], in_=gkv_d[0].ap())
        for s in range(2):
            t = xpool.tile([128, 2 * PADF], BF16, tag="x", name="x")
            nc.sync.dma_start(out=t[:],
                              in_=xp_d[s].ap()[0].rearrange("p c f -> p (c f)"))
            xt[(0, s, 0)] = t
        nc.sync.dma_start(out=gq_sb[0][:], in_=gq_d[0].ap())
        nc.sync.dma_start(out=gkv_sb[1][:], in_=gkv_d[1].ap())
        nc.sync.dma_start(out=gq_sb[1][:], in_=gq_d[1].ap())
        t = xpool.tile([128, PADF], BF16, tag="xq", name="xq")
        nc.sync.dma_start(out=t[:], in_=xq_d.ap()[0])
        xt[(0, 1, 1)] = t
        dqs_sb = const.tile([128, 9], F32, tag="dqs", name="dqs")
        nc.sync.dma_start(out=dqs_sb[:], in_=dqs_d.ap())
        for s in range(2):
            bq_sb.append(const.tile([128, 4], F32, tag=f"bq{s}", name=f"bq{s}"))
            bk_sb.append(const.tile([128, 4], F32, tag=f"bk{s}", name=f"bk{s}"))
            bo_sb.append(const.tile([128, 2], F32, tag=f"bo{s}", name=f"bo{s}"))
            nc.sync.dma_start(out=bq_sb[s][:], in_=bq_d[s].ap())
            nc.sync.dma_start(out=bk_sb[s][:], in_=bk_d[s].ap())
            nc.sync.dma_start(out=bo_sb[s][:], in_=bo_d[s].ap())
        ones_sb = const.tile([128, 64], BF16, tag="ones", name="ones")
        nc.sync.dma_start(out=ones_sb[:], in_=ones_d.ap())
        for s in range(2):
            t = xpool.tile([128, 2 * PADF], BF16, tag="x", name="x")
            nc.sync.dma_start(out=t[:],
                              in_=xp_d[s].ap()[1].rearrange("p c f -> p (c f)"))
            xt[(1, s, 0)] = t
        t = xpool.tile([128, PADF], BF16, tag="xq", name="xq")
        nc.sync.dma_start(out=t[:], in_=xq_d.ap()[1])
        xt[(1, 1, 1)] = t
        # big pointwise weights on the gpsimd-issued queue, in parallel
        for s in range(2):
            wkv_sb.append(const.tile([128, 2, 1024], BF16, tag=f"wkv{s}",
                                     name=f"wkv{s}"))
            nc.gpsimd.dma_start(out=wkv_sb[s][:], in_=wkv_d[s].ap())
            wq_sb.append(const.tile([128, 2, 512], BF16, tag=f"wq{s}", name=f"wq{s}"))
            nc.gpsimd.dma_start(out=wq_sb[s][:], in_=wq_d[s].ap())
        for s in range(2):
            wo_sb.append(const.tile([128, 4, 256], BF16, tag=f"wo{s}", name=f"wo{s}"))
            nc.gpsimd.dma_start(out=wo_sb[s][:], in_=wo_d[s].ap())

        st = [dict(yq={}, ykv={}, qT={}, kT={}, v={}) for _ in range(IMGS)]

        TAPS = [(ky, kx) for ky in range(3) for kx in range(3)]

        # ---- emission helpers ---------------------------------------------
        def kvdw(img, s):
            """stride-2 depthwise on PE via diagonal weights, both chunks."""
            xv = xt[(img, s, 0)][:].rearrange("p (c r q) -> p c r q", c=2, r=PADW)
            for c in range(2):
                ps = psD.tile([128, HWK], F32, tag="d", name="d")
                for t, (ky, kx) in enumerate(TAPS):
                    nc.tensor.matmul(
                        out=ps[:],
                        lhsT=gkv_sb[s][:, c, t, :],
                        rhs=xv[:, c, ky:ky + 32:2, kx:kx + 32:2],
                        start=(t == 0), stop=(t == 8))
                yk = ykpool.tile([128, HWK], BF16, tag="ykv", name="ykv")
                nc.vector.tensor_copy(yk[:], ps[:])
                st[img]['ykv'][(s, c)] = yk

        def qdw_pe(img, s, c):
            """stride-1 depthwise on PE via diagonal weights (one chunk)."""
            xv = xt[(img, s, 0)][:].rearrange("p (c r q) -> p c r q", c=2, r=PADW)
            yq_t = yqpool.tile([128, HW], BF16, tag="yq", name="yq")
            for h in range(2):
                ps = psA.tile([128, 512], F32, tag="mm", name="mm")
                for t, (ky, kx) in enumerate(TAPS):
                    nc.tensor.matmul(
                        out=ps[:],
                        lhsT=gq_sb[s][:, c, t, :],
                        rhs=xv[:, c, ky + 16 * h:ky + 16 * h + 16, kx:kx + 32],
                        start=(t == 0), stop=(t == 8))
                nc.vector.tensor_copy(yq_t[:, h * 512:(h + 1) * 512], ps[:])
            st[img]['yq'][(s, c)] = yq_t

        def qdw_dve(img, s, c):
            """stride-1 depthwise on DVE (needs the column-shifted x copy)."""
            xv = xt[(img, s, 0)][:].rearrange("p (c r q) -> p c r q", c=2, r=PADW)
            sv = xt[(img, s, 1)][:].rearrange("p (r q) -> p r q", r=PADW)
            yq_t = yqpool.tile([128, HW], BF16, tag="yq", name="yq")
            y3 = yq_t[:].rearrange("p (r q) -> p r q", r=32)
            nc.vector.tensor_scalar_mul(y3[:, :, :], sv[:, 1:33, 0:32],
                                        dqs_sb[:, 4:5])
            for t, (ky, kx) in enumerate(TAPS):
                if t == 4:
                    continue
                in0 = (sv[:, ky:ky + 32, 0:32] if kx == 1
                       else xv[:, c, ky:ky + 32, kx:kx + 32])
                nc.vector.scalar_tensor_tensor(
                    out=y3[:, :, :], in0=in0, scalar=dqs_sb[:, t:t + 1],
                    in1=y3[:, :, :], op0=AOP.mult, op1=AOP.add)
            st[img]['yq'][(s, c)] = yq_t

        def proj_kv(img, s):
            ykv = st[img]['ykv']
            # K pointwise: kT[m] [128 cout, 256 kvpos]; bias fused on ACT
            for m in range(4):
                ps = psA.tile([128, HWK], F32, tag="mm", name="mm")
                for k in range(2):
                    nc.tensor.matmul(
                        out=ps[:],
                        lhsT=wkv_sb[s][:, k, m * 128:(m + 1) * 128],
                        rhs=ykv[(s, k)][:],
                        start=(k == 0), stop=(k == 1))
                kT_t = kTpool.tile([128, HWK], BF16, tag="kT", name="kT")
                nc.scalar.activation(out=kT_t[:], in_=ps[:], func=AF.Identity,
                                     bias=bk_sb[s][:, m:m + 1], scale=1.0)
                st[img]['kT'][(s, m)] = kT_t
            # V pointwise (x-stationary): v[p] [128 kvpos, 512 cout]
            for p in range(2):
                ps = psA.tile([128, 512], F32, tag="mm", name="mm")
                for k in range(2):
                    nc.tensor.matmul(
                        out=ps[:],
                        lhsT=ykv[(s, k)][:, p * 128:(p + 1) * 128],
                        rhs=wkv_sb[s][:, k, 512:1024],
                        start=(k == 0), stop=(k == 1))
                v_t = vpool.tile([128, 512], BF16, tag="v", name="v")
                nc.vector.tensor_copy(v_t[:], ps[:])
                st[img]['v'][(s, p)] = v_t

        def proj_q(img, s, ms):
            yq = st[img]['yq']
            for m in ms:
                qT_t = qTpool.tile([128, HW], BF16, tag="qT", name="qT")
                for qh in range(2):
                    ps = psA.tile([128, 512], F32, tag="mm", name="mm")
                    for k in range(2):
                        nc.tensor.matmul(
                            out=ps[:],
                            lhsT=wq_sb[s][:, k, m * 128:(m + 1) * 128],
                            rhs=yq[(s, k)][:, qh * 512:(qh + 1) * 512],
                            start=(k == 0), stop=(k == 1))
                    nc.vector.tensor_scalar_add(
                        qT_t[:, qh * 512:(qh + 1) * 512], ps[:],
                        bq_sb[s][:, m:m + 1])
                st[img]['qT'][(s, m)] = qT_t

        def dwproj_packets(img):
            pkts = []
            for s in (0, 1):
                pkts.append(lambda s=s: kvdw(img, s))
                pkts.append(lambda s=s: proj_kv(img, s))
                for c in (0, 1):
                    if (s, c) in DW_DVE:
                        pkts.append(lambda s=s, c=c: qdw_dve(img, s, c))
                    else:
                        pkts.append(lambda s=s, c=c: qdw_pe(img, s, c))
                pkts.append(lambda s=s: proj_q(img, s, (0, 1)))
                pkts.append(lambda s=s: proj_q(img, s, (2, 3)))
            return pkts

        def dots_hp(img, a, hp, pT):
            """dots + exp for one head-pair (fills pT)."""
            b = 1 - a
            qT, kT = st[img]['qT'], st[img]['kT']
            for kc in range(2):
                dp = [psB.tile([128, HW], F32, tag="big", name="big")
                      for _ in range(2)]
                for qh in range(2):
                    for j in range(2):
                        nc.tensor.matmul(
                            out=dp[j][:, qh * 512:(qh + 1) * 512],
                            lhsT=kT[(b, hp)][64 * j:64 * (j + 1),
                                             kc * 128:(kc + 1) * 128],
                            rhs=qT[(a, hp)][64 * j:64 * (j + 1),
                                            qh * 512:(qh + 1) * 512],
                            start=True, stop=True)
                for j in range(2):
                    pT_t = pTpool.tile([128, HW], BF16, tag="pT", name="pT")
                    nc.scalar.activation(out=pT_t[:], in_=dp[j][:],
                                         func=AF.Exp, scale=SCALE)
                    pT[(2 * hp + j, kc)] = pT_t

        def soft_hp(img, a, hp, pT):
            """denominators + attn@v + normalized oT for one head-pair."""
            b = 1 - a
            v = st[img]['v']
            oT_t = oTpool.tile([128, HW], BF16, tag="oT", name="oT")
            for qh in range(2):
                d_t = psD.tile([128, 512], F32, tag="d", name="d")
                for kc in range(2):
                    nc.tensor.matmul(
                        out=d_t[0:64, :], lhsT=ones_sb[:],
                        rhs=pT[(2 * hp, kc)][:, qh * 512:(qh + 1) * 512],
                        start=(kc == 0), stop=(kc == 1), skip_group_check=True)
                    nc.tensor.matmul(
                        out=d_t[64:128, :], lhsT=ones_sb[:],
                        rhs=pT[(2 * hp + 1, kc)][:, qh * 512:(qh + 1) * 512],
                        start=(kc == 0), stop=(kc == 1), skip_group_check=True)
                dr_t = Rpool.tile([128, 512], F32, tag="R", name="R")
                nc.vector.reciprocal_approx_fast(out=dr_t[:], in_=d_t[:])
                po = psD.tile([128, 512], F32, tag="d", name="d")
                for kc in range(2):
                    nc.tensor.matmul(
                        out=po[0:64, :],
                        lhsT=v[(b, kc)][:, 128 * hp:128 * hp + 64],
                        rhs=pT[(2 * hp, kc)][:, qh * 512:(qh + 1) * 512],
                        start=(kc == 0), stop=(kc == 1), skip_group_check=True)
                    nc.tensor.matmul(
                        out=po[64:128, :],
                        lhsT=v[(b, kc)][:, 128 * hp + 64:128 * (hp + 1)],
                        rhs=pT[(2 * hp + 1, kc)][:, qh * 512:(qh + 1) * 512],
                        start=(kc == 0), stop=(kc == 1), skip_group_check=True)
                nc.vector.tensor_mul(oT_t[:, qh * 512:(qh + 1) * 512],
                                     po[:], dr_t[:])
            return oT_t

        def attn_out(img, a, oT):
            for cc in range(2):
                out_t = outpool.tile([128, HW], BF16, tag="out", name="out")
                for qh in range(2):
                    ps = psA.tile([128, 512], F32, tag="mm", name="mm")
                    for hp in range(4):
                        nc.tensor.matmul(
                            out=ps[:],
                            lhsT=wo_sb[a][:, hp, cc * 128:(cc + 1) * 128],
                            rhs=oT[hp][:, qh * 512:(qh + 1) * 512],
                            start=(hp == 0), stop=(hp == 3))
                    nc.vector.tensor_scalar_add(
                        out_t[:, qh * 512:(qh + 1) * 512], ps[:],
                        bo_sb[a][:, cc:cc + 1])
                nc.gpsimd.dma_start(
                    out=out_d.ap()[a, img, cc * 128:(cc + 1) * 128, :],
                    in_=out_t[:])

        def attn_packets(img):
            """per direction: dots(hp+1) emitted before soft(hp) so the PE
            queue always has independent work between dependent stages."""
            pkts = []
            for a in range(2):
                pT, oT = {}, {}
                pkts.append(lambda a=a, pT=pT: dots_hp(img, a, 0, pT))
                for hp in range(4):
                    def step(a=a, hp=hp, pT=pT, oT=oT):
                        if hp < 3:
                            dots_hp(img, a, hp + 1, pT)
                        oT[hp] = soft_hp(img, a, hp, pT)
                    pkts.append(step)
                pkts.append(lambda a=a, oT=oT: attn_out(img, a, oT))
            return pkts

        # ---- drive: img0 dwproj, then img0 attention interleaved with
        # img1 dwproj, then img1 attention ----------------------------------
        for p in dwproj_packets(0):
            p()
        fg = attn_packets(0)
        bg = dwproj_packets(1)
        done = 0
        for i, p in enumerate(fg):
            p()
            want = (i + 1) * len(bg) // len(fg)
            while done < want:
                bg[done]()
                done += 1
        while done < len(bg):
            bg[done]()
            done += 1
        for p in attn_packets(1):
            p()

    nc.compile()
    return nc


_MODULE = None


def _get_module():
    global _MODULE
    if _MODULE is None:
        _MODULE = _build_module()
    return _MODULE


# ---------------------------------------------------------------------------
# host side: BN folding + padding + sharding + launch
# ---------------------------------------------------------------------------

def _fold(inputs, p):
    dw = np.asarray(inputs[p + '_dw'], np.float32)[:, 0]        # [256,3,3]
    g = np.asarray(inputs[p + '_g'], np.float32)
    b_ = np.asarray(inputs[p + '_b'], np.float32)
    rm = np.asarray(inputs[p + '_rm'], np.float32)
    rv = np.asarray(inputs[p + '_rv'], np.float32)
    pw = np.asarray(inputs[p + '_pw'], np.float32)[:, :, 0, 0]  # [cout, 256]
    inv = g / np.sqrt(rv + EPS)
    dw_eff = (dw * inv[:, None, None]).reshape(DIM, 9)
    bias = pw @ (b_ - rm * inv)
    return dw_eff, pw.T.copy(), bias                             # WT [256, cout]


def _diag(dw_eff):
    """dw_eff [256, 9] -> [128, 2, 9, 128] bf16 diagonal weight tiles."""
    g = np.zeros((128, 2, 9, 128), np.float32)
    ii = np.arange(128)
    for c in range(2):
        g[ii, c, :, ii] = dw_eff[c * 128:(c + 1) * 128, :]
    return g.astype(BF)


def host_arrays(inputs):
    """Folded per-core-constant DRAM tensors (same on every core)."""
    h = {'ones_in': np.ones((128, 64), BF)}
    bv = {}
    for s, qp, kvp in ((0, 'q1', 'kv1'), (1, 'q2', 'kv2')):
        dwq, WqT, bq = _fold(inputs, qp)
        dwkv, WkvT, bkv = _fold(inputs, kvp)
        h[f'gq{s}'] = _diag(dwq)
        h[f'gkv{s}'] = _diag(dwkv)
        s_dve, c_dve = next(iter(DW_DVE))
        if s == s_dve:
            h['dqs1'] = np.ascontiguousarray(
                dwq[c_dve * 128:(c_dve + 1) * 128, :])
        h[f'wq{s}'] = np.ascontiguousarray(
            WqT.reshape(2, 128, 512).transpose(1, 0, 2)).astype(BF)
        h[f'wkv{s}'] = np.ascontiguousarray(
            WkvT.reshape(2, 128, 2 * 512).transpose(1, 0, 2)).astype(BF)
        h[f'bq{s}'] = np.ascontiguousarray(bq.reshape(4, 128).T)
        h[f'bk{s}'] = np.ascontiguousarray(bkv[:INNER].reshape(4, 128).T)
        bv[s] = bkv[INNER:]
    for s, op in ((0, 'out1'), (1, 'out2')):
        Wout = np.asarray(inputs[op + '_w'], np.float32)[:, :, 0, 0]  # [256, 512]
        bo = np.asarray(inputs[op + '_b'], np.float32) + Wout @ bv[1 - s]
        h[f'wo{s}'] = np.ascontiguousarray(
            Wout.T.reshape(4, 128, 256).transpose(1, 0, 2)).astype(BF)
        h[f'bo{s}'] = np.ascontiguousarray(bo.reshape(2, 128).T)
    for k, a in h.items():
        if a.dtype != BF:
            h[k] = np.ascontiguousarray(a, dtype=np.float32)
    return h


def _pad_x(x):
    """x [B, 256, 32, 32] -> padded [B, 128, 2, 34, 34] bf16, interior (1,1)."""
    xb = x.astype(BF).reshape(B, 2, 128, 32, 32).transpose(0, 2, 1, 3, 4)
    xp = np.zeros((B, 128, 2, PADW, PADW), BF)
    xp[:, :, :, 1:33, 1:33] = xb
    return xp.reshape(B, 128, 2, PADF)


def make_in_maps(inputs):
    h = host_arrays(inputs)
    x1 = np.asarray(inputs['x1'], np.float32)
    x2 = np.asarray(inputs['x2'], np.float32)
    xp0 = _pad_x(x1)
    xp1 = _pad_x(x2)
    # column-shifted copy of x2's chunk-1 for the DVE depthwise unit
    s_dve, c_dve = next(iter(DW_DVE))
    xsrc = (x2 if s_dve else x1).astype(BF)
    xs = np.zeros((B, 128, PADW, PADW), BF)
    xs[:, :, 1:33, 0:32] = xsrc.reshape(B, 2, 128, 32, 32)[:, c_dve]
    xq1 = xs.reshape(B, 128, PADF)
    maps = []
    for c in range(N_CORES):
        sl = slice(c * IMGS, (c + 1) * IMGS)
        m = dict(h)
        m['xp0'] = np.ascontiguousarray(xp0[sl])
        m['xp1'] = np.ascontiguousarray(xp1[sl])
        m['xq1'] = np.ascontiguousarray(xq1[sl])
        maps.append(m)
    return maps


def gather_out(core_outs):
    """core_outs: list of [2, IMGS, 256, 1024] bf16 -> [2, B, 256, 32, 32]."""
    full = np.concatenate([np.asarray(o).astype(np.float32) for o in core_outs],
                          axis=1)
    return np.ascontiguousarray(full.reshape(2, B, DIM, 32, 32))


def kernel(**inputs):
    nc = _get_module()
    in_maps = make_in_maps(inputs)
    res = run_bass_kernel_spmd(nc, in_maps, list(range(N_CORES)))
    return gather_out([r['out'] for r in res.results])


if __name__ == '__main__':
    nc = _build_module()
    print("module built OK")


# revision 8
# speedup vs baseline: 1.1234x; 1.1234x over previous
"""ConvCrossAttention Trainium2 kernel (Bass/Tile), SPMD over 8 NeuronCores.

Sharding: pure data-parallel over batch (B=16 -> 2 images per core). Each core
runs the full two-stream cross-attention block for its 2 images; no collectives.

v3. bf16 matmul path (fp32 PSUM accumulation) with the work split to balance
engines (scalar_tensor_tensor has no 2x DVE uop, so depthwise on DVE runs 1x):
  - depthwise convs run on the PE as 9 accumulating diagonal-weight matmuls
    over a host-padded 34x34 zero-border x (dense strided rhs views, stride-2
    views for the KV path). One of the four Q-side (stream, chunk) units stays
    on the DVE (via a column-shifted x copy so its taps are step-1 aligned) to
    keep PE/DVE occupancy balanced at ~105us each.
  - attention dots are K=64 row-tiled pairs (tile_position auto-derived from
    base_partition 0/64) so head pairs run concurrently in the PE array;
    softmax denominator and attn@v use M=64 col-tiled pairs (PSUM partition
    slices 0:64 / 64:128).
  - exp on ACT straight off the dots PSUM into bf16 pT; denominators
    reciprocal'd on DVE; normalization fused into the attn@v eviction.
  - image 1's depthwise+projections are emitted interleaved with image 0's
    attention blocks so DVE/PE/ACT phases overlap instead of serializing.
  - output is written bf16 (host upcasts); K-proj bias eviction on ACT,
    Q-proj/out-conv bias evictions on DVE.
"""

import numpy as np
import ml_dtypes
from contextlib import ExitStack

import concourse.bass as bass
import concourse.bacc as bacc
import concourse.tile as tile
import concourse.mybir as mybir
from concourse.bass_utils import run_bass_kernel_spmd

F32 = mybir.dt.float32
BF16 = mybir.dt.bfloat16
AOP = mybir.AluOpType
AF = mybir.ActivationFunctionType

N_CORES = 8
B = 16
IMGS = B // N_CORES          # images per core
DIM = 256                    # conv in channels
HEADS, DH, INNER = 8, 64, 512
HW = 1024                    # 32*32 q positions
HWK = 256                    # 16*16 kv positions
EPS = 1e-5
SCALE = DH ** -0.5
PADW = 34                    # zero-padded spatial extent
PADF = PADW * PADW           # 1156

BF = ml_dtypes.bfloat16

DW_DVE = {(1, 1)}            # Q-depthwise (stream, chunk) units kept on DVE


# ---------------------------------------------------------------------------
# device kernel
# ---------------------------------------------------------------------------

def _build_module():
    nc = bacc.Bacc("TRN2", target_bir_lowering=False, debug=False)

    xp_d = [nc.dram_tensor(f"xp{s}", [IMGS, 128, 2, PADF], BF16, kind="ExternalInput")
            for s in range(2)]
    xq_d = nc.dram_tensor("xq1", [IMGS, 128, PADF], BF16, kind="ExternalInput")
    dqs_d = nc.dram_tensor("dqs1", [128, 9], F32, kind="ExternalInput")
    gq_d = [nc.dram_tensor(f"gq{s}", [128, 2, 9, 128], BF16, kind="ExternalInput")
            for s in range(2)]
    gkv_d = [nc.dram_tensor(f"gkv{s}", [128, 2, 9, 128], BF16, kind="ExternalInput")
             for s in range(2)]
    wq_d = [nc.dram_tensor(f"wq{s}", [128, 2, 512], BF16, kind="ExternalInput")
            for s in range(2)]
    wkv_d = [nc.dram_tensor(f"wkv{s}", [128, 2, 1024], BF16, kind="ExternalInput")
             for s in range(2)]
    wo_d = [nc.dram_tensor(f"wo{s}", [128, 4, 256], BF16, kind="ExternalInput")
            for s in range(2)]
    bq_d = [nc.dram_tensor(f"bq{s}", [128, 4], F32, kind="ExternalInput")
            for s in range(2)]
    bk_d = [nc.dram_tensor(f"bk{s}", [128, 4], F32, kind="ExternalInput")
            for s in range(2)]
    bo_d = [nc.dram_tensor(f"bo{s}", [128, 2], F32, kind="ExternalInput")
            for s in range(2)]
    ones_d = nc.dram_tensor("ones_in", [128, 64], BF16, kind="ExternalInput")
    out_d = nc.dram_tensor("out", [2, IMGS, DIM, HW], BF16, kind="ExternalOutput")

    with tile.TileContext(nc) as tc, ExitStack() as ctx:
        const = ctx.enter_context(tc.tile_pool(name="const", bufs=1))
        xpool = ctx.enter_context(tc.tile_pool(name="xpool", bufs=6))
        yqpool = ctx.enter_context(tc.tile_pool(name="yqpool", bufs=6))
        ykpool = ctx.enter_context(tc.tile_pool(name="ykpool", bufs=6))
        qTpool = ctx.enter_context(tc.tile_pool(name="qTpool", bufs=16))
        kTpool = ctx.enter_context(tc.tile_pool(name="kTpool", bufs=16))
        vpool = ctx.enter_context(tc.tile_pool(name="vpool", bufs=8))
        pTpool = ctx.enter_context(tc.tile_pool(name="pTpool", bufs=10))
        Rpool = ctx.enter_context(tc.tile_pool(name="Rpool", bufs=3))
        oTpool = ctx.enter_context(tc.tile_pool(name="oTpool", bufs=6))
        outpool = ctx.enter_context(tc.tile_pool(name="outpool", bufs=3))
        psA = ctx.enter_context(tc.tile_pool(name="psA", bufs=2, space="PSUM"))
        psB = ctx.enter_context(tc.tile_pool(name="psB", bufs=3, space="PSUM"))

        # ---- constants + x loads (order = DMA priority) --------------------
        gq_sb, gkv_sb, wq_sb, wkv_sb, wo_sb, bq_sb, bk_sb, bo_sb = (
            [], [], [], [], [], [], [], [])
        for s in range(2):
            gkv_sb.append(const.tile([128, 2, 9, 128], BF16, tag=f"gkv{s}",
                                     name=f"gkv{s}"))
            gq_sb.append(const.tile([128, 2, 9, 128], BF16, tag=f"gq{s}",
                                    name=f"gq{s}"))
        xt = {}
        # first image's x + stream-0 diag weights lead the queue so the PE can
        # start depthwise almost immediately
        nc.sync.dma_start(out=gkv_sb[0][:], in_=gkv_d[0].ap())
        for s in range(2):
            t = xpool.tile([128, 2 * PADF], BF16, tag="x", name="x")
            nc.sync.dma_start(out=t[:],
                              in_=xp_d[s].ap()[0].rearrange("p c f -> p (c f)"))
            xt[(0, s, 0)] = t
        nc.sync.dma_start(out=gq_sb[0][:], in_=gq_d[0].ap())
        nc.sync.dma_start(out=gkv_sb[1][:], in_=gkv_d[1].ap())
        nc.sync.dma_start(out=gq_sb[1][:], in_=gq_d[1].ap())
        t = xpool.tile([128, PADF], BF16, tag="xq", name="xq")
        nc.sync.dma_start(out=t[:], in_=xq_d.ap()[0])
        xt[(0, 1, 1)] = t
        dqs_sb = const.tile([128, 9], F32, tag="dqs", name="dqs")
        nc.sync.dma_start(out=dqs_sb[:], in_=dqs_d.ap())
        for s in range(2):
            bq_sb.append(const.tile([128, 4], F32, tag=f"bq{s}", name=f"bq{s}"))
            bk_sb.append(const.tile([128, 4], F32, tag=f"bk{s}", name=f"bk{s}"))
            bo_sb.append(const.tile([128, 2], F32, tag=f"bo{s}", name=f"bo{s}"))
            nc.sync.dma_start(out=bq_sb[s][:], in_=bq_d[s].ap())
            nc.sync.dma_start(out=bk_sb[s][:], in_=bk_d[s].ap())
            nc.sync.dma_start(out=bo_sb[s][:], in_=bo_d[s].ap())
        ones_sb = const.tile([128, 64], BF16, tag="ones", name="ones")
        nc.sync.dma_start(out=ones_sb[:], in_=ones_d.ap())
        for s in range(2):
            t = xpool.tile([128, 2 * PADF], BF16, tag="x", name="x")
            nc.sync.dma_start(out=t[:],
                              in_=xp_d[s].ap()[1].rearrange("p c f -> p (c f)"))
            xt[(1, s, 0)] = t
        t = xpool.tile([128, PADF], BF16, tag="xq", name="xq")
        nc.sync.dma_start(out=t[:], in_=xq_d.ap()[1])
        xt[(1, 1, 1)] = t
        # big pointwise weights on the gpsimd-issued queue, in parallel
        for s in range(2):
            wkv_sb.append(const.tile([128, 2, 1024], BF16, tag=f"wkv{s}",
                                     name=f"wkv{s}"))
            nc.gpsimd.dma_start(out=wkv_sb[s][:], in_=wkv_d[s].ap())
            wq_sb.append(const.tile([128, 2, 512], BF16, tag=f"wq{s}", name=f"wq{s}"))
            nc.gpsimd.dma_start(out=wq_sb[s][:], in_=wq_d[s].ap())
        for s in range(2):
            wo_sb.append(const.tile([128, 4, 256], BF16, tag=f"wo{s}", name=f"wo{s}"))
            nc.gpsimd.dma_start(out=wo_sb[s][:], in_=wo_d[s].ap())

        st = [dict(yq={}, ykv={}, qT={}, kT={}, v={}) for _ in range(IMGS)]

        TAPS = [(ky, kx) for ky in range(3) for kx in range(3)]

        # ---- emission helpers ---------------------------------------------
        def kvdw(img, s):
            """stride-2 depthwise on PE via diagonal weights, both chunks."""
            xv = xt[(img, s, 0)][:].rearrange("p (c r q) -> p c r q", c=2, r=PADW)
            for c in range(2):
                ps = psA.tile([128, HWK], F32, tag="mm", name="mm")
                for t, (ky, kx) in enumerate(TAPS):
                    nc.tensor.matmul(
                        out=ps[:],
                        lhsT=gkv_sb[s][:, c, t, :],
                        rhs=xv[:, c, ky:ky + 32:2, kx:kx + 32:2],
                        start=(t == 0), stop=(t == 8))
                yk = ykpool.tile([128, HWK], BF16, tag="ykv", name="ykv")
                nc.vector.tensor_copy(yk[:], ps[:])
                st[img]['ykv'][(s, c)] = yk

        def qdw_pe(img, s, c):
            """stride-1 depthwise on PE via diagonal weights (one chunk)."""
            xv = xt[(img, s, 0)][:].rearrange("p (c r q) -> p c r q", c=2, r=PADW)
            yq_t = yqpool.tile([128, HW], BF16, tag="yq", name="yq")
            for h in range(2):
                ps = psA.tile([128, 512], F32, tag="mm", name="mm")
                for t, (ky, kx) in enumerate(TAPS):
                    nc.tensor.matmul(
                        out=ps[:],
                        lhsT=gq_sb[s][:, c, t, :],
                        rhs=xv[:, c, ky + 16 * h:ky + 16 * h + 16, kx:kx + 32],
                        start=(t == 0), stop=(t == 8))
                nc.vector.tensor_copy(yq_t[:, h * 512:(h + 1) * 512], ps[:])
            st[img]['yq'][(s, c)] = yq_t

        def qdw_dve(img, s, c):
            """stride-1 depthwise on DVE (needs the column-shifted x copy)."""
            xv = xt[(img, s, 0)][:].rearrange("p (c r q) -> p c r q", c=2, r=PADW)
            sv = xt[(img, s, 1)][:].rearrange("p (r q) -> p r q", r=PADW)
            yq_t = yqpool.tile([128, HW], BF16, tag="yq", name="yq")
            y3 = yq_t[:].rearrange("p (r q) -> p r q", r=32)
            nc.vector.tensor_scalar_mul(y3[:, :, :], sv[:, 1:33, 0:32],
                                        dqs_sb[:, 4:5])
            for t, (ky, kx) in enumerate(TAPS):
                if t == 4:
                    continue
                in0 = (sv[:, ky:ky + 32, 0:32] if kx == 1
                       else xv[:, c, ky:ky + 32, kx:kx + 32])
                nc.vector.scalar_tensor_tensor(
                    out=y3[:, :, :], in0=in0, scalar=dqs_sb[:, t:t + 1],
                    in1=y3[:, :, :], op0=AOP.mult, op1=AOP.add)
            st[img]['yq'][(s, c)] = yq_t

        def proj_kv(img, s):
            ykv = st[img]['ykv']
            # K pointwise: kT[m] [128 cout, 256 kvpos]; bias fused on ACT
            for m in range(4):
                ps = psA.tile([128, HWK], F32, tag="mm", name="mm")
                for k in range(2):
                    nc.tensor.matmul(
                        out=ps[:],
                        lhsT=wkv_sb[s][:, k, m * 128:(m + 1) * 128],
                        rhs=ykv[(s, k)][:],
                        start=(k == 0), stop=(k == 1))
                kT_t = kTpool.tile([128, HWK], BF16, tag="kT", name="kT")
                nc.scalar.activation(out=kT_t[:], in_=ps[:], func=AF.Identity,
                                     bias=bk_sb[s][:, m:m + 1], scale=1.0)
                st[img]['kT'][(s, m)] = kT_t
            # V pointwise (x-stationary): v[p] [128 kvpos, 512 cout]
            for p in range(2):
                ps = psA.tile([128, 512], F32, tag="mm", name="mm")
                for k in range(2):
                    nc.tensor.matmul(
                        out=ps[:],
                        lhsT=ykv[(s, k)][:, p * 128:(p + 1) * 128],
                        rhs=wkv_sb[s][:, k, 512:1024],
                        start=(k == 0), stop=(k == 1))
                v_t = vpool.tile([128, 512], BF16, tag="v", name="v")
                nc.vector.tensor_copy(v_t[:], ps[:])
                st[img]['v'][(s, p)] = v_t

        def proj_q(img, s, ms):
            yq = st[img]['yq']
            for m in ms:
                qT_t = qTpool.tile([128, HW], BF16, tag="qT", name="qT")
                for qh in range(2):
                    ps = psA.tile([128, 512], F32, tag="mm", name="mm")
                    for k in range(2):
                        nc.tensor.matmul(
                            out=ps[:],
                            lhsT=wq_sb[s][:, k, m * 128:(m + 1) * 128],
                            rhs=yq[(s, k)][:, qh * 512:(qh + 1) * 512],
                            start=(k == 0), stop=(k == 1))
                    nc.vector.tensor_scalar_add(
                        qT_t[:, qh * 512:(qh + 1) * 512], ps[:],
                        bq_sb[s][:, m:m + 1])
                st[img]['qT'][(s, m)] = qT_t

        def dwproj_packets(img):
            pkts = []
            for s in (0, 1):
                pkts.append(lambda s=s: kvdw(img, s))
                pkts.append(lambda s=s: proj_kv(img, s))
                for c in (0, 1):
                    if (s, c) in DW_DVE:
                        pkts.append(lambda s=s, c=c: qdw_dve(img, s, c))
                    else:
                        pkts.append(lambda s=s, c=c: qdw_pe(img, s, c))
                pkts.append(lambda s=s: proj_q(img, s, (0, 1)))
                pkts.append(lambda s=s: proj_q(img, s, (2, 3)))
            return pkts

        def dots_hp(img, a, hp, pT):
            """dots + exp for one head-pair (fills pT)."""
            b = 1 - a
            qT, kT = st[img]['qT'], st[img]['kT']
            for kc in range(2):
                dp = [psB.tile([128, HW], F32, tag="big", name="big")
                      for _ in range(2)]
                for qh in range(2):
                    for j in range(2):
                        nc.tensor.matmul(
                            out=dp[j][:, qh * 512:(qh + 1) * 512],
                            lhsT=kT[(b, hp)][64 * j:64 * (j + 1),
                                             kc * 128:(kc + 1) * 128],
                            rhs=qT[(a, hp)][64 * j:64 * (j + 1),
                                            qh * 512:(qh + 1) * 512],
                            start=True, stop=True)
                for j in range(2):
                    pT_t = pTpool.tile([128, HW], BF16, tag="pT", name="pT")
                    nc.scalar.activation(out=pT_t[:], in_=dp[j][:],
                                         func=AF.Exp, scale=SCALE)
                    pT[(2 * hp + j, kc)] = pT_t

        def soft_hp(img, a, hp, pT):
            """denominators + attn@v + normalized oT for one head-pair.
            attn@v sits between the two denominator halves so each
            reciprocal hides under PE work; po holds a psB slot so the next
            head-pair's dots can run concurrently in the other two."""
            b = 1 - a
            v = st[img]['v']
            oT_t = oTpool.tile([128, HW], BF16, tag="oT", name="oT")
            dr_t = Rpool.tile([128, HW], F32, tag="R", name="R")
            po = psB.tile([128, HW], F32, tag="big", name="big")

            def denom(qh):
                d_t = psA.tile([128, 512], F32, tag="mm", name="mm")
                for kc in range(2):
                    nc.tensor.matmul(
                        out=d_t[0:64, :], lhsT=ones_sb[:],
                        rhs=pT[(2 * hp, kc)][:, qh * 512:(qh + 1) * 512],
                        start=(kc == 0), stop=(kc == 1), skip_group_check=True)
                    nc.tensor.matmul(
                        out=d_t[64:128, :], lhsT=ones_sb[:],
                        rhs=pT[(2 * hp + 1, kc)][:, qh * 512:(qh + 1) * 512],
                        start=(kc == 0), stop=(kc == 1), skip_group_check=True)
                nc.vector.reciprocal_approx_fast(
                    out=dr_t[:, qh * 512:(qh + 1) * 512], in_=d_t[:])

            denom(0)
            for qh in range(2):
                for kc in range(2):
                    nc.tensor.matmul(
                        out=po[0:64, qh * 512:(qh + 1) * 512],
                        lhsT=v[(b, kc)][:, 128 * hp:128 * hp + 64],
                        rhs=pT[(2 * hp, kc)][:, qh * 512:(qh + 1) * 512],
                        start=(kc == 0), stop=(kc == 1), skip_group_check=True)
                    nc.tensor.matmul(
                        out=po[64:128, qh * 512:(qh + 1) * 512],
                        lhsT=v[(b, kc)][:, 128 * hp + 64:128 * (hp + 1)],
                        rhs=pT[(2 * hp + 1, kc)][:, qh * 512:(qh + 1) * 512],
                        start=(kc == 0), stop=(kc == 1), skip_group_check=True)
            denom(1)
            nc.vector.tensor_mul(oT_t[:], po[:], dr_t[:])
            return oT_t

        def attn_out(img, a, oT):
            for cc in range(2):
                out_t = outpool.tile([128, HW], BF16, tag="out", name="out")
                for qh in range(2):
                    ps = psA.tile([128, 512], F32, tag="mm", name="mm")
                    for hp in range(4):
                        nc.tensor.matmul(
                            out=ps[:],
                            lhsT=wo_sb[a][:, hp, cc * 128:(cc + 1) * 128],
                            rhs=oT[hp][:, qh * 512:(qh + 1) * 512],
                            start=(hp == 0), stop=(hp == 3))
                    nc.vector.tensor_scalar_add(
                        out_t[:, qh * 512:(qh + 1) * 512], ps[:],
                        bo_sb[a][:, cc:cc + 1])
                nc.gpsimd.dma_start(
                    out=out_d.ap()[a, img, cc * 128:(cc + 1) * 128, :],
                    in_=out_t[:])

        def attn_packets(img):
            """per direction: dots(hp+1) emitted before soft(hp) so the PE
            queue always has independent work between dependent stages."""
            pkts = []
            for a in range(2):
                pT, oT = {}, {}
                pkts.append(lambda a=a, pT=pT: dots_hp(img, a, 0, pT))
                for hp in range(4):
                    def step(a=a, hp=hp, pT=pT, oT=oT):
                        if hp < 3:
                            dots_hp(img, a, hp + 1, pT)
                        oT[hp] = soft_hp(img, a, hp, pT)
                    pkts.append(step)
                pkts.append(lambda a=a, oT=oT: attn_out(img, a, oT))
            return pkts

        # ---- drive: img0 dwproj, then img0 attention interleaved with
        # img1 dwproj, then img1 attention ----------------------------------
        for p in dwproj_packets(0):
            p()
        fg = attn_packets(0)
        bg = dwproj_packets(1)
        done = 0
        for i, p in enumerate(fg):
            p()
            want = (i + 1) * len(bg) // len(fg)
            while done < want:
                bg[done]()
                done += 1
        while done < len(bg):
            bg[done]()
            done += 1
        for p in attn_packets(1):
            p()

    nc.compile()
    return nc


_MODULE = None


def _get_module():
    global _MODULE
    if _MODULE is None:
        _MODULE = _build_module()
    return _MODULE


# ---------------------------------------------------------------------------
# host side: BN folding + padding + sharding + launch
# ---------------------------------------------------------------------------

def _fold(inputs, p):
    dw = np.asarray(inputs[p + '_dw'], np.float32)[:, 0]        # [256,3,3]
    g = np.asarray(inputs[p + '_g'], np.float32)
    b_ = np.asarray(inputs[p + '_b'], np.float32)
    rm = np.asarray(inputs[p + '_rm'], np.float32)
    rv = np.asarray(inputs[p + '_rv'], np.float32)
    pw = np.asarray(inputs[p + '_pw'], np.float32)[:, :, 0, 0]  # [cout, 256]
    inv = g / np.sqrt(rv + EPS)
    dw_eff = (dw * inv[:, None, None]).reshape(DIM, 9)
    bias = pw @ (b_ - rm * inv)
    return dw_eff, pw.T.copy(), bias                             # WT [256, cout]


def _diag(dw_eff):
    """dw_eff [256, 9] -> [128, 2, 9, 128] bf16 diagonal weight tiles."""
    g = np.zeros((128, 2, 9, 128), np.float32)
    ii = np.arange(128)
    for c in range(2):
        g[ii, c, :, ii] = dw_eff[c * 128:(c + 1) * 128, :]
    return g.astype(BF)


def host_arrays(inputs):
    """Folded per-core-constant DRAM tensors (same on every core)."""
    h = {'ones_in': np.ones((128, 64), BF)}
    bv = {}
    for s, qp, kvp in ((0, 'q1', 'kv1'), (1, 'q2', 'kv2')):
        dwq, WqT, bq = _fold(inputs, qp)
        dwkv, WkvT, bkv = _fold(inputs, kvp)
        h[f'gq{s}'] = _diag(dwq)
        h[f'gkv{s}'] = _diag(dwkv)
        s_dve, c_dve = next(iter(DW_DVE))
        if s == s_dve:
            h['dqs1'] = np.ascontiguousarray(
                dwq[c_dve * 128:(c_dve + 1) * 128, :])
        h[f'wq{s}'] = np.ascontiguousarray(
            WqT.reshape(2, 128, 512).transpose(1, 0, 2)).astype(BF)
        h[f'wkv{s}'] = np.ascontiguousarray(
            WkvT.reshape(2, 128, 2 * 512).transpose(1, 0, 2)).astype(BF)
        h[f'bq{s}'] = np.ascontiguousarray(bq.reshape(4, 128).T)
        h[f'bk{s}'] = np.ascontiguousarray(bkv[:INNER].reshape(4, 128).T)
        bv[s] = bkv[INNER:]
    for s, op in ((0, 'out1'), (1, 'out2')):
        Wout = np.asarray(inputs[op + '_w'], np.float32)[:, :, 0, 0]  # [256, 512]
        bo = np.asarray(inputs[op + '_b'], np.float32) + Wout @ bv[1 - s]
        h[f'wo{s}'] = np.ascontiguousarray(
            Wout.T.reshape(4, 128, 256).transpose(1, 0, 2)).astype(BF)
        h[f'bo{s}'] = np.ascontiguousarray(bo.reshape(2, 128).T)
    for k, a in h.items():
        if a.dtype != BF:
            h[k] = np.ascontiguousarray(a, dtype=np.float32)
    return h


def _pad_x(x):
    """x [B, 256, 32, 32] -> padded [B, 128, 2, 34, 34] bf16, interior (1,1)."""
    xb = x.astype(BF).reshape(B, 2, 128, 32, 32).transpose(0, 2, 1, 3, 4)
    xp = np.zeros((B, 128, 2, PADW, PADW), BF)
    xp[:, :, :, 1:33, 1:33] = xb
    return xp.reshape(B, 128, 2, PADF)


def make_in_maps(inputs):
    h = host_arrays(inputs)
    x1 = np.asarray(inputs['x1'], np.float32)
    x2 = np.asarray(inputs['x2'], np.float32)
    xp0 = _pad_x(x1)
    xp1 = _pad_x(x2)
    # column-shifted copy of x2's chunk-1 for the DVE depthwise unit
    s_dve, c_dve = next(iter(DW_DVE))
    xsrc = (x2 if s_dve else x1).astype(BF)
    xs = np.zeros((B, 128, PADW, PADW), BF)
    xs[:, :, 1:33, 0:32] = xsrc.reshape(B, 2, 128, 32, 32)[:, c_dve]
    xq1 = xs.reshape(B, 128, PADF)
    maps = []
    for c in range(N_CORES):
        sl = slice(c * IMGS, (c + 1) * IMGS)
        m = dict(h)
        m['xp0'] = np.ascontiguousarray(xp0[sl])
        m['xp1'] = np.ascontiguousarray(xp1[sl])
        m['xq1'] = np.ascontiguousarray(xq1[sl])
        maps.append(m)
    return maps


def gather_out(core_outs):
    """core_outs: list of [2, IMGS, 256, 1024] bf16 -> [2, B, 256, 32, 32]."""
    full = np.concatenate([np.asarray(o).astype(np.float32) for o in core_outs],
                          axis=1)
    return np.ascontiguousarray(full.reshape(2, B, DIM, 32, 32))


def kernel(**inputs):
    nc = _get_module()
    in_maps = make_in_maps(inputs)
    res = run_bass_kernel_spmd(nc, in_maps, list(range(N_CORES)))
    return gather_out([r['out'] for r in res.results])


if __name__ == '__main__':
    nc = _build_module()
    print("module built OK")


# revision 9
# speedup vs baseline: 1.1601x; 1.0327x over previous
"""ConvCrossAttention Trainium2 kernel (Bass/Tile), SPMD over 8 NeuronCores.

Sharding: pure data-parallel over batch (B=16 -> 2 images per core). Each core
runs the full two-stream cross-attention block for its 2 images; no collectives.

v3. bf16 matmul path (fp32 PSUM accumulation) with the work split to balance
engines (scalar_tensor_tensor has no 2x DVE uop, so depthwise on DVE runs 1x):
  - depthwise convs run on the PE as 9 accumulating diagonal-weight matmuls
    over a host-padded 34x34 zero-border x (dense strided rhs views, stride-2
    views for the KV path). One of the four Q-side (stream, chunk) units stays
    on the DVE (via a column-shifted x copy so its taps are step-1 aligned) to
    keep PE/DVE occupancy balanced at ~105us each.
  - attention dots are K=64 row-tiled pairs (tile_position auto-derived from
    base_partition 0/64) so head pairs run concurrently in the PE array;
    softmax denominator and attn@v use M=64 col-tiled pairs (PSUM partition
    slices 0:64 / 64:128).
  - exp on ACT straight off the dots PSUM into bf16 pT; denominators
    reciprocal'd on DVE; normalization fused into the attn@v eviction.
  - image 1's depthwise+projections are emitted interleaved with image 0's
    attention blocks so DVE/PE/ACT phases overlap instead of serializing.
  - output is written bf16 (host upcasts); K-proj bias eviction on ACT,
    Q-proj/out-conv bias evictions on DVE.
"""

import numpy as np
import ml_dtypes
from contextlib import ExitStack

import concourse.bass as bass
import concourse.bacc as bacc
import concourse.tile as tile
import concourse.mybir as mybir
from concourse.bass_utils import run_bass_kernel_spmd

F32 = mybir.dt.float32
BF16 = mybir.dt.bfloat16
AOP = mybir.AluOpType
AF = mybir.ActivationFunctionType

N_CORES = 8
B = 16
IMGS = B // N_CORES          # images per core
DIM = 256                    # conv in channels
HEADS, DH, INNER = 8, 64, 512
HW = 1024                    # 32*32 q positions
HWK = 256                    # 16*16 kv positions
EPS = 1e-5
SCALE = DH ** -0.5
PADW = 34                    # zero-padded spatial extent
PADF = PADW * PADW           # 1156

BF = ml_dtypes.bfloat16

DW_DVE = {(1, 1)}            # Q-depthwise (stream, chunk) units kept on DVE


# ---------------------------------------------------------------------------
# device kernel
# ---------------------------------------------------------------------------

def _build_module():
    nc = bacc.Bacc("TRN2", target_bir_lowering=False, debug=False)

    xp_d = [nc.dram_tensor(f"xp{s}", [IMGS, 128, 2, PADF], BF16, kind="ExternalInput")
            for s in range(2)]
    xq_d = nc.dram_tensor("xq1", [IMGS, 128, PADF], BF16, kind="ExternalInput")
    dqs_d = nc.dram_tensor("dqs1", [128, 9], F32, kind="ExternalInput")
    gq_d = [nc.dram_tensor(f"gq{s}", [128, 2, 9, 128], BF16, kind="ExternalInput")
            for s in range(2)]
    gkv_d = [nc.dram_tensor(f"gkv{s}", [128, 2, 9, 128], BF16, kind="ExternalInput")
             for s in range(2)]
    wq_d = [nc.dram_tensor(f"wq{s}", [128, 2, 512], BF16, kind="ExternalInput")
            for s in range(2)]
    wkv_d = [nc.dram_tensor(f"wkv{s}", [128, 2, 1024], BF16, kind="ExternalInput")
             for s in range(2)]
    wo_d = [nc.dram_tensor(f"wo{s}", [128, 4, 256], BF16, kind="ExternalInput")
            for s in range(2)]
    bq_d = [nc.dram_tensor(f"bq{s}", [128, 4], F32, kind="ExternalInput")
            for s in range(2)]
    bk_d = [nc.dram_tensor(f"bk{s}", [128, 4], F32, kind="ExternalInput")
            for s in range(2)]
    bo_d = [nc.dram_tensor(f"bo{s}", [128, 2], F32, kind="ExternalInput")
            for s in range(2)]
    ones_d = nc.dram_tensor("ones_in", [128, 64], BF16, kind="ExternalInput")
    out_d = nc.dram_tensor("out", [2, IMGS, DIM, HW], BF16, kind="ExternalOutput")

    with tile.TileContext(nc) as tc, ExitStack() as ctx:
        const = ctx.enter_context(tc.tile_pool(name="const", bufs=1))
        xpool = ctx.enter_context(tc.tile_pool(name="xpool", bufs=10))
        yqpool = ctx.enter_context(tc.tile_pool(name="yqpool", bufs=6))
        ykpool = ctx.enter_context(tc.tile_pool(name="ykpool", bufs=6))
        qTpool = ctx.enter_context(tc.tile_pool(name="qTpool", bufs=16))
        kTpool = ctx.enter_context(tc.tile_pool(name="kTpool", bufs=16))
        vpool = ctx.enter_context(tc.tile_pool(name="vpool", bufs=8))
        pTpool = ctx.enter_context(tc.tile_pool(name="pTpool", bufs=10))
        Rpool = ctx.enter_context(tc.tile_pool(name="Rpool", bufs=3))
        oTpool = ctx.enter_context(tc.tile_pool(name="oTpool", bufs=6))
        outpool = ctx.enter_context(tc.tile_pool(name="outpool", bufs=3))
        psA = ctx.enter_context(tc.tile_pool(name="psA", bufs=2, space="PSUM"))
        psB = ctx.enter_context(tc.tile_pool(name="psB", bufs=3, space="PSUM"))

        # ---- constants + x loads (order = DMA priority) --------------------
        # tiny bias/scalar tensors first (they gate PSUM evictions), then
        # diag weights + x per chunk, interleaved so the first depthwise
        # matmul can start as early as possible.
        gq_sb, gkv_sb, wq_sb, wkv_sb, wo_sb, bq_sb, bk_sb, bo_sb = (
            [], [], [], [], [], [], [], [])
        for s in range(2):
            gkv_sb.append([const.tile([128, 9, 128], BF16, tag=f"gkv{s}{c}",
                                      name=f"gkv{s}{c}") for c in range(2)])
            gq_sb.append([const.tile([128, 9, 128], BF16, tag=f"gq{s}{c}",
                                     name=f"gq{s}{c}") for c in range(2)])
            bq_sb.append(const.tile([128, 4], F32, tag=f"bq{s}", name=f"bq{s}"))
            bk_sb.append(const.tile([128, 4], F32, tag=f"bk{s}", name=f"bk{s}"))
            bo_sb.append(const.tile([128, 2], F32, tag=f"bo{s}", name=f"bo{s}"))
        ones_sb = const.tile([128, 64], BF16, tag="ones", name="ones")
        dqs_sb = const.tile([128, 9], F32, tag="dqs", name="dqs")
        for s in range(2):
            nc.sync.dma_start(out=bq_sb[s][:], in_=bq_d[s].ap())
            nc.sync.dma_start(out=bk_sb[s][:], in_=bk_d[s].ap())
            nc.sync.dma_start(out=bo_sb[s][:], in_=bo_d[s].ap())
        nc.sync.dma_start(out=ones_sb[:], in_=ones_d.ap())
        nc.sync.dma_start(out=dqs_sb[:], in_=dqs_d.ap())
        xt = {}

        def load_x(img, s, c):
            t = xpool.tile([128, PADF], BF16, tag="x", name="x")
            nc.sync.dma_start(out=t[:], in_=xp_d[s].ap()[img, :, c, :])
            xt[(img, s, c)] = t

        for c in range(2):
            nc.sync.dma_start(out=gkv_sb[0][c][:], in_=gkv_d[0].ap()[:, c])
            load_x(0, 0, c)
        for c in range(2):
            nc.sync.dma_start(out=gq_sb[0][c][:], in_=gq_d[0].ap()[:, c])
        for c in range(2):
            nc.sync.dma_start(out=gkv_sb[1][c][:], in_=gkv_d[1].ap()[:, c])
            load_x(0, 1, c)
            nc.sync.dma_start(out=gq_sb[1][c][:], in_=gq_d[1].ap()[:, c])
        t = xpool.tile([128, PADF], BF16, tag="x", name="x")
        nc.sync.dma_start(out=t[:], in_=xq_d.ap()[0])
        xt[(0, 'sh')] = t
        for s in range(2):
            for c in range(2):
                load_x(1, s, c)
        t = xpool.tile([128, PADF], BF16, tag="x", name="x")
        nc.sync.dma_start(out=t[:], in_=xq_d.ap()[1])
        xt[(1, 'sh')] = t
        # big pointwise weights on the gpsimd-issued queue, in parallel
        for s in range(2):
            wkv_sb.append(const.tile([128, 2, 1024], BF16, tag=f"wkv{s}",
                                     name=f"wkv{s}"))
            nc.gpsimd.dma_start(out=wkv_sb[s][:], in_=wkv_d[s].ap())
            wq_sb.append(const.tile([128, 2, 512], BF16, tag=f"wq{s}", name=f"wq{s}"))
            nc.gpsimd.dma_start(out=wq_sb[s][:], in_=wq_d[s].ap())
        for s in range(2):
            wo_sb.append(const.tile([128, 4, 256], BF16, tag=f"wo{s}", name=f"wo{s}"))
            nc.gpsimd.dma_start(out=wo_sb[s][:], in_=wo_d[s].ap())

        st = [dict(yq={}, ykv={}, qT={}, kT={}, v={}) for _ in range(IMGS)]

        TAPS = [(ky, kx) for ky in range(3) for kx in range(3)]

        # ---- emission helpers ---------------------------------------------
        def kvdw(img, s):
            """stride-2 depthwise on PE via diagonal weights, both chunks."""
            for c in range(2):
                xv = xt[(img, s, c)][:].rearrange("p (r q) -> p r q", r=PADW)
                ps = psA.tile([128, HWK], F32, tag="mm", name="mm")
                for t, (ky, kx) in enumerate(TAPS):
                    nc.tensor.matmul(
                        out=ps[:],
                        lhsT=gkv_sb[s][c][:, t, :],
                        rhs=xv[:, ky:ky + 32:2, kx:kx + 32:2],
                        start=(t == 0), stop=(t == 8))
                yk = ykpool.tile([128, HWK], BF16, tag="ykv", name="ykv")
                nc.vector.tensor_copy(yk[:], ps[:])
                st[img]['ykv'][(s, c)] = yk

        def qdw_pe(img, s, c):
            """stride-1 depthwise on PE via diagonal weights (one chunk)."""
            xv = xt[(img, s, c)][:].rearrange("p (r q) -> p r q", r=PADW)
            yq_t = yqpool.tile([128, HW], BF16, tag="yq", name="yq")
            for h in range(2):
                ps = psA.tile([128, 512], F32, tag="mm", name="mm")
                for t, (ky, kx) in enumerate(TAPS):
                    nc.tensor.matmul(
                        out=ps[:],
                        lhsT=gq_sb[s][c][:, t, :],
                        rhs=xv[:, ky + 16 * h:ky + 16 * h + 16, kx:kx + 32],
                        start=(t == 0), stop=(t == 8))
                nc.vector.tensor_copy(yq_t[:, h * 512:(h + 1) * 512], ps[:])
            st[img]['yq'][(s, c)] = yq_t

        def qdw_dve(img, s, c):
            """stride-1 depthwise on DVE (needs the column-shifted x copy)."""
            xv = xt[(img, s, c)][:].rearrange("p (r q) -> p r q", r=PADW)
            sv = xt[(img, 'sh')][:].rearrange("p (r q) -> p r q", r=PADW)
            yq_t = yqpool.tile([128, HW], BF16, tag="yq", name="yq")
            y3 = yq_t[:].rearrange("p (r q) -> p r q", r=32)
            nc.vector.tensor_scalar_mul(y3[:, :, :], sv[:, 1:33, 0:32],
                                        dqs_sb[:, 4:5])
            for t, (ky, kx) in enumerate(TAPS):
                if t == 4:
                    continue
                in0 = (sv[:, ky:ky + 32, 0:32] if kx == 1
                       else xv[:, ky:ky + 32, kx:kx + 32])
                nc.vector.scalar_tensor_tensor(
                    out=y3[:, :, :], in0=in0, scalar=dqs_sb[:, t:t + 1],
                    in1=y3[:, :, :], op0=AOP.mult, op1=AOP.add)
            st[img]['yq'][(s, c)] = yq_t

        def proj_kv(img, s):
            ykv = st[img]['ykv']
            # K pointwise: kT[m] [128 cout, 256 kvpos]; bias fused on ACT
            for m in range(4):
                ps = psA.tile([128, HWK], F32, tag="mm", name="mm")
                for k in range(2):
                    nc.tensor.matmul(
                        out=ps[:],
                        lhsT=wkv_sb[s][:, k, m * 128:(m + 1) * 128],
                        rhs=ykv[(s, k)][:],
                        start=(k == 0), stop=(k == 1))
                kT_t = kTpool.tile([128, HWK], BF16, tag="kT", name="kT")
                nc.scalar.activation(out=kT_t[:], in_=ps[:], func=AF.Identity,
                                     bias=bk_sb[s][:, m:m + 1], scale=1.0)
                st[img]['kT'][(s, m)] = kT_t
            # V pointwise (x-stationary): v[p] [128 kvpos, 512 cout]
            for p in range(2):
                ps = psA.tile([128, 512], F32, tag="mm", name="mm")
                for k in range(2):
                    nc.tensor.matmul(
                        out=ps[:],
                        lhsT=ykv[(s, k)][:, p * 128:(p + 1) * 128],
                        rhs=wkv_sb[s][:, k, 512:1024],
                        start=(k == 0), stop=(k == 1))
                v_t = vpool.tile([128, 512], BF16, tag="v", name="v")
                nc.vector.tensor_copy(v_t[:], ps[:])
                st[img]['v'][(s, p)] = v_t

        def proj_q(img, s, ms):
            yq = st[img]['yq']
            for m in ms:
                qT_t = qTpool.tile([128, HW], BF16, tag="qT", name="qT")
                for qh in range(2):
                    ps = psA.tile([128, 512], F32, tag="mm", name="mm")
                    for k in range(2):
                        nc.tensor.matmul(
                            out=ps[:],
                            lhsT=wq_sb[s][:, k, m * 128:(m + 1) * 128],
                            rhs=yq[(s, k)][:, qh * 512:(qh + 1) * 512],
                            start=(k == 0), stop=(k == 1))
                    nc.vector.tensor_scalar_add(
                        qT_t[:, qh * 512:(qh + 1) * 512], ps[:],
                        bq_sb[s][:, m:m + 1])
                st[img]['qT'][(s, m)] = qT_t

        def dwproj_packets(img):
            pkts = []
            for s in (0, 1):
                pkts.append(lambda s=s: kvdw(img, s))
                pkts.append(lambda s=s: proj_kv(img, s))
                for c in (0, 1):
                    if (s, c) in DW_DVE:
                        pkts.append(lambda s=s, c=c: qdw_dve(img, s, c))
                    else:
                        pkts.append(lambda s=s, c=c: qdw_pe(img, s, c))
                pkts.append(lambda s=s: proj_q(img, s, (0, 1)))
                pkts.append(lambda s=s: proj_q(img, s, (2, 3)))
            return pkts

        def dots_hp(img, a, hp, pT):
            """dots + exp for one head-pair (fills pT)."""
            b = 1 - a
            qT, kT = st[img]['qT'], st[img]['kT']
            for kc in range(2):
                dp = [psB.tile([128, HW], F32, tag="big", name="big")
                      for _ in range(2)]
                for qh in range(2):
                    for j in range(2):
                        nc.tensor.matmul(
                            out=dp[j][:, qh * 512:(qh + 1) * 512],
                            lhsT=kT[(b, hp)][64 * j:64 * (j + 1),
                                             kc * 128:(kc + 1) * 128],
                            rhs=qT[(a, hp)][64 * j:64 * (j + 1),
                                            qh * 512:(qh + 1) * 512],
                            start=True, stop=True)
                for j in range(2):
                    pT_t = pTpool.tile([128, HW], BF16, tag="pT", name="pT")
                    nc.scalar.activation(out=pT_t[:], in_=dp[j][:],
                                         func=AF.Exp, scale=SCALE)
                    pT[(2 * hp + j, kc)] = pT_t

        def soft_hp(img, a, hp, pT):
            """denominators + attn@v + normalized oT for one head-pair.
            attn@v sits between the two denominator halves so each
            reciprocal hides under PE work; po holds a psB slot so the next
            head-pair's dots can run concurrently in the other two."""
            b = 1 - a
            v = st[img]['v']
            oT_t = oTpool.tile([128, HW], BF16, tag="oT", name="oT")
            dr_t = Rpool.tile([128, HW], F32, tag="R", name="R")
            po = psB.tile([128, HW], F32, tag="big", name="big")

            def denom(qh):
                d_t = psA.tile([128, 512], F32, tag="mm", name="mm")
                for kc in range(2):
                    nc.tensor.matmul(
                        out=d_t[0:64, :], lhsT=ones_sb[:],
                        rhs=pT[(2 * hp, kc)][:, qh * 512:(qh + 1) * 512],
                        start=(kc == 0), stop=(kc == 1), skip_group_check=True)
                    nc.tensor.matmul(
                        out=d_t[64:128, :], lhsT=ones_sb[:],
                        rhs=pT[(2 * hp + 1, kc)][:, qh * 512:(qh + 1) * 512],
                        start=(kc == 0), stop=(kc == 1), skip_group_check=True)
                nc.vector.reciprocal_approx_fast(
                    out=dr_t[:, qh * 512:(qh + 1) * 512], in_=d_t[:])

            denom(0)
            for qh in range(2):
                for kc in range(2):
                    nc.tensor.matmul(
                        out=po[0:64, qh * 512:(qh + 1) * 512],
                        lhsT=v[(b, kc)][:, 128 * hp:128 * hp + 64],
                        rhs=pT[(2 * hp, kc)][:, qh * 512:(qh + 1) * 512],
                        start=(kc == 0), stop=(kc == 1), skip_group_check=True)
                    nc.tensor.matmul(
                        out=po[64:128, qh * 512:(qh + 1) * 512],
                        lhsT=v[(b, kc)][:, 128 * hp + 64:128 * (hp + 1)],
                        rhs=pT[(2 * hp + 1, kc)][:, qh * 512:(qh + 1) * 512],
                        start=(kc == 0), stop=(kc == 1), skip_group_check=True)
            denom(1)
            nc.vector.tensor_mul(oT_t[:], po[:], dr_t[:])
            return oT_t

        def attn_out(img, a, oT):
            for cc in range(2):
                out_t = outpool.tile([128, HW], BF16, tag="out", name="out")
                for qh in range(2):
                    ps = psA.tile([128, 512], F32, tag="mm", name="mm")
                    for hp in range(4):
                        nc.tensor.matmul(
                            out=ps[:],
                            lhsT=wo_sb[a][:, hp, cc * 128:(cc + 1) * 128],
                            rhs=oT[hp][:, qh * 512:(qh + 1) * 512],
                            start=(hp == 0), stop=(hp == 3))
                    nc.vector.tensor_scalar_add(
                        out_t[:, qh * 512:(qh + 1) * 512], ps[:],
                        bo_sb[a][:, cc:cc + 1])
                nc.gpsimd.dma_start(
                    out=out_d.ap()[a, img, cc * 128:(cc + 1) * 128, :],
                    in_=out_t[:])

        def attn_packets(img):
            """per direction: dots(hp+1) emitted before soft(hp) so the PE
            queue always has independent work between dependent stages."""
            pkts = []
            for a in range(2):
                pT, oT = {}, {}
                pkts.append(lambda a=a, pT=pT: dots_hp(img, a, 0, pT))
                for hp in range(4):
                    def step(a=a, hp=hp, pT=pT, oT=oT):
                        if hp < 3:
                            dots_hp(img, a, hp + 1, pT)
                        oT[hp] = soft_hp(img, a, hp, pT)
                    pkts.append(step)
                pkts.append(lambda a=a, oT=oT: attn_out(img, a, oT))
            return pkts

        # ---- drive: img0 dwproj, then img0 attention interleaved with
        # img1 dwproj, then img1 attention ----------------------------------
        for p in dwproj_packets(0):
            p()
        fg = attn_packets(0)
        bg = dwproj_packets(1)
        done = 0
        for i, p in enumerate(fg):
            p()
            want = (i + 1) * len(bg) // len(fg)
            while done < want:
                bg[done]()
                done += 1
        while done < len(bg):
            bg[done]()
            done += 1
        for p in attn_packets(1):
            p()

    nc.compile()
    return nc


_MODULE = None


def _get_module():
    global _MODULE
    if _MODULE is None:
        _MODULE = _build_module()
    return _MODULE


# ---------------------------------------------------------------------------
# host side: BN folding + padding + sharding + launch
# ---------------------------------------------------------------------------

def _fold(inputs, p):
    dw = np.asarray(inputs[p + '_dw'], np.float32)[:, 0]        # [256,3,3]
    g = np.asarray(inputs[p + '_g'], np.float32)
    b_ = np.asarray(inputs[p + '_b'], np.float32)
    rm = np.asarray(inputs[p + '_rm'], np.float32)
    rv = np.asarray(inputs[p + '_rv'], np.float32)
    pw = np.asarray(inputs[p + '_pw'], np.float32)[:, :, 0, 0]  # [cout, 256]
    inv = g / np.sqrt(rv + EPS)
    dw_eff = (dw * inv[:, None, None]).reshape(DIM, 9)
    bias = pw @ (b_ - rm * inv)
    return dw_eff, pw.T.copy(), bias                             # WT [256, cout]


def _diag(dw_eff):
    """dw_eff [256, 9] -> [128, 2, 9, 128] bf16 diagonal weight tiles."""
    g = np.zeros((128, 2, 9, 128), np.float32)
    ii = np.arange(128)
    for c in range(2):
        g[ii, c, :, ii] = dw_eff[c * 128:(c + 1) * 128, :]
    return g.astype(BF)


def host_arrays(inputs):
    """Folded per-core-constant DRAM tensors (same on every core)."""
    h = {'ones_in': np.ones((128, 64), BF)}
    bv = {}
    for s, qp, kvp in ((0, 'q1', 'kv1'), (1, 'q2', 'kv2')):
        dwq, WqT, bq = _fold(inputs, qp)
        dwkv, WkvT, bkv = _fold(inputs, kvp)
        h[f'gq{s}'] = _diag(dwq)
        h[f'gkv{s}'] = _diag(dwkv)
        s_dve, c_dve = next(iter(DW_DVE))
        if s == s_dve:
            h['dqs1'] = np.ascontiguousarray(
                dwq[c_dve * 128:(c_dve + 1) * 128, :])
        h[f'wq{s}'] = np.ascontiguousarray(
            WqT.reshape(2, 128, 512).transpose(1, 0, 2)).astype(BF)
        h[f'wkv{s}'] = np.ascontiguousarray(
            WkvT.reshape(2, 128, 2 * 512).transpose(1, 0, 2)).astype(BF)
        h[f'bq{s}'] = np.ascontiguousarray(bq.reshape(4, 128).T)
        h[f'bk{s}'] = np.ascontiguousarray(bkv[:INNER].reshape(4, 128).T)
        bv[s] = bkv[INNER:]
    for s, op in ((0, 'out1'), (1, 'out2')):
        Wout = np.asarray(inputs[op + '_w'], np.float32)[:, :, 0, 0]  # [256, 512]
        bo = np.asarray(inputs[op + '_b'], np.float32) + Wout @ bv[1 - s]
        h[f'wo{s}'] = np.ascontiguousarray(
            Wout.T.reshape(4, 128, 256).transpose(1, 0, 2)).astype(BF)
        h[f'bo{s}'] = np.ascontiguousarray(bo.reshape(2, 128).T)
    for k, a in h.items():
        if a.dtype != BF:
            h[k] = np.ascontiguousarray(a, dtype=np.float32)
    return h


def _pad_x(x):
    """x [B, 256, 32, 32] -> padded [B, 128, 2, 34, 34] bf16, interior (1,1)."""
    xb = x.astype(BF).reshape(B, 2, 128, 32, 32).transpose(0, 2, 1, 3, 4)
    xp = np.zeros((B, 128, 2, PADW, PADW), BF)
    xp[:, :, :, 1:33, 1:33] = xb
    return xp.reshape(B, 128, 2, PADF)


def make_in_maps(inputs):
    h = host_arrays(inputs)
    x1 = np.asarray(inputs['x1'], np.float32)
    x2 = np.asarray(inputs['x2'], np.float32)
    xp0 = _pad_x(x1)
    xp1 = _pad_x(x2)
    # column-shifted copy of x2's chunk-1 for the DVE depthwise unit
    s_dve, c_dve = next(iter(DW_DVE))
    xsrc = (x2 if s_dve else x1).astype(BF)
    xs = np.zeros((B, 128, PADW, PADW), BF)
    xs[:, :, 1:33, 0:32] = xsrc.reshape(B, 2, 128, 32, 32)[:, c_dve]
    xq1 = xs.reshape(B, 128, PADF)
    maps = []
    for c in range(N_CORES):
        sl = slice(c * IMGS, (c + 1) * IMGS)
        m = dict(h)
        m['xp0'] = np.ascontiguousarray(xp0[sl])
        m['xp1'] = np.ascontiguousarray(xp1[sl])
        m['xq1'] = np.ascontiguousarray(xq1[sl])
        maps.append(m)
    return maps


def gather_out(core_outs):
    """core_outs: list of [2, IMGS, 256, 1024] bf16 -> [2, B, 256, 32, 32]."""
    full = np.concatenate([np.asarray(o).astype(np.float32) for o in core_outs],
                          axis=1)
    return np.ascontiguousarray(full.reshape(2, B, DIM, 32, 32))


def kernel(**inputs):
    nc = _get_module()
    in_maps = make_in_maps(inputs)
    res = run_bass_kernel_spmd(nc, in_maps, list(range(N_CORES)))
    return gather_out([r['out'] for r in res.results])


if __name__ == '__main__':
    nc = _build_module()
    print("module built OK")
